# revision 44
# baseline (speedup 1.0000x reference)
"""ANOVA kernel (order 3) on 8 TRN2 NeuronCores — 1KiB-descriptor variant.

Layout per core: tile tau covers 16 consecutive b. SBUF tile (128, 512)
bf16: partition p = b_g*16 + fq (b-oct b_g in [0,8), f-quad fq in
[0,16)), free n = j3*256 + par*64 + e (par in [0,4) is f mod 4). DRAM
element offset = p*256 + j3*32768 + par*64 + e: 1-KiB contiguous DMA
descriptors. 4 PSUM fills alternating 2 bank sets; finale per fill.
"""

import sys

if "/opt/trn_rl_repo" not in sys.path:
    sys.path.insert(0, "/opt/trn_rl_repo")

import numpy as np

N_CORES = 8
B, F, E = 8192, 64, 64
B_PER_CORE = B // N_CORES  # 1024
J3 = 2
PAR = 4
FD = 512
TILES = B_PER_CORE // 16   # 64
SUPER = 4
N_SUPER = TILES // SUPER   # 16
SFD = FD * SUPER           # 2048
FILL_SUPERS = 4
N_FILLS = N_SUPER // FILL_SUPERS  # 4
DVE_SQUARE_SUPERS = {4, 12}

_cache = {}


def _make_g() -> np.ndarray:
    import ml_dtypes

    g = np.zeros((128, 120), dtype=ml_dtypes.bfloat16)
    for k in range(128):
        g[k, 56 + k // 16] = 1.0
    return g


def _build():
    import concourse.bass as bass
    import concourse.tile as tile
    from concourse import bacc, mybir

    nc = bacc.Bacc(
        "TRN2", target_bir_lowering=False, debug=False, num_devices=N_CORES
    )
    f32 = mybir.dt.float32
    bf16 = mybir.dt.bfloat16

    x_dram = nc.dram_tensor(
        "x", [B_PER_CORE, F, E], f32, kind="ExternalInput"
    ).ap()
    g_dram = nc.dram_tensor("g", [128, 120], bf16, kind="ExternalInput").ap()
    out_dram = nc.dram_tensor(
        "out", [128, N_FILLS * J3], f32, kind="ExternalOutput"
    ).ap()

    TILE_ELEMS = 16 * F * E  # 65536

    def x_ap(tile0: int, ntiles: int) -> bass.AP:
        ap = [[PAR * E, 128]]
        if ntiles > 1:
            ap.append([TILE_ELEMS, ntiles])
        ap += [[8 * F * E, J3], [1, PAR * E]]
        return bass.AP(tensor=x_dram.tensor, offset=tile0 * TILE_ELEMS, ap=ap)

    with tile.TileContext(nc) as tc:
        with (
            tc.tile_pool(name="const", bufs=1) as const_pool,
            tc.tile_pool(name="xin", bufs=12) as x_pool,
            tc.tile_pool(name="xsq", bufs=5) as x2_pool,
            tc.tile_pool(name="xcu", bufs=5) as x3_pool,
            tc.tile_pool(name="acc", bufs=1, space="PSUM") as psum_pool,
            tc.tile_pool(name="tail", bufs=2) as tail_pool,
        ):
            g_sb = const_pool.tile([128, 120], bf16)
            nc.sync.dma_start(out=g_sb[:], in_=g_dram[:])
            outt = const_pool.tile([128, N_FILLS * J3], f32)

            psums = [
                [
                    psum_pool.tile([128, FD], f32, name=f"psum_{st}_{stat}")
                    for stat in range(3)
                ]
                for st in range(2)
            ]

            def finale(phi: int):
                p1t, p2t, p3t = psums[phi % 2]
                pa = []
                for idx, pt in enumerate((p1t, p2t, p3t)):
                    v = pt[:].rearrange("p (j t e) -> p j t e", j=J3, t=PAR)
                    a = tail_pool.tile([128, J3, E], f32, name=f"pa{idx}")
                    nc.scalar.copy(a[:], v[:, :, 0, :])
                    for t in range(1, PAR):
                        nc.vector.tensor_add(a[:], a[:], v[:, :, t, :])
                    pa.append(a)
                pa1, pa2, pa3 = pa
                HFD = J3 * E
                t1 = tail_pool.tile([128, HFD], f32)
                nc.scalar.square(t1[:], pa1[:])
                u2 = tail_pool.tile([128, HFD], f32)
                nc.vector.scalar_tensor_tensor(
                    u2[:], pa2[:], -3.0, t1[:],
                    op0=mybir.AluOpType.mult, op1=mybir.AluOpType.add,
                )
                u3 = tail_pool.tile([128, HFD], f32)
                nc.vector.tensor_mul(u3[:], u2[:], pa1[:])
                u5 = tail_pool.tile([128, HFD], f32)
                nc.vector.scalar_tensor_tensor(
                    u5[:], pa3[:], 2.0, u3[:],
                    op0=mybir.AluOpType.mult, op1=mybir.AluOpType.add,
                )
                red = tail_pool.tile([128, J3], f32)
                nc.vector.reduce_sum(
                    red[:],
                    u5[:].rearrange("p (j e) -> p j e", j=J3),
                    axis=mybir.AxisListType.X,
                )
                cols = slice(J3 * phi, J3 * (phi + 1))
                nc.vector.tensor_scalar_mul(outt[:, cols], red[:], 1.0 / 6.0)
                nc.sync.dma_start(out=out_dram[:, cols], in_=outt[:, cols])

            for s in range(N_SUPER):
                xb = x_pool.tile([128, SFD], bf16)
                nsplit = (
                    SUPER if s == 0 else 2 if s in (1, N_SUPER - 1) else 1
                )
                csz = SFD // nsplit
                for c in range(nsplit):
                    nc.gpsimd.dma_start(
                        out=xb[:, c * csz : (c + 1) * csz],
                        in_=x_ap(
                            s * SUPER + c * (SUPER // nsplit), SUPER // nsplit
                        ),
                    )
                x2b = x2_pool.tile([128, SFD], bf16)
                x3b = x3_pool.tile([128, SFD], bf16)
                for c in range(nsplit):
                    cs = slice(c * csz, (c + 1) * csz)
                    if s in DVE_SQUARE_SUPERS:
                        nc.vector.tensor_mul(x2b[:, cs], xb[:, cs], xb[:, cs])
                    else:
                        nc.scalar.square(x2b[:, cs], xb[:, cs])
                for k in range(SUPER):
                    ks = slice(k * FD, (k + 1) * FD)
                    nc.vector.tensor_mul(x3b[:, ks], x2b[:, ks], xb[:, ks])
                phi = s // FILL_SUPERS
                for stat, src in enumerate((xb, x2b, x3b)):
                    psum = psums[phi % 2][stat]
                    for k in range(SUPER):
                        taupp = (s % FILL_SUPERS) * SUPER + k
                        m8, cg = taupp % 8, taupp // 8
                        nc.tensor.matmul(
                            psum[64 * cg : 64 * cg + 64, :],
                            g_sb[:, 56 - 8 * m8 : 120 - 8 * m8],
                            src[:, k * FD : (k + 1) * FD],
                            start=m8 == 0,
                            stop=m8 == 7,
                            skip_group_check=True,
                        )
                if s % FILL_SUPERS == FILL_SUPERS - 1:
                    finale(s // FILL_SUPERS)

    nc.compile()
    return nc


def _get_nc():
    if "nc" not in _cache:
        _cache["nc"] = _build()
    return _cache["nc"]


def _unpermute(r: np.ndarray) -> np.ndarray:
    return np.transpose(
        r.reshape(16, 8, N_FILLS, J3), (2, 0, 3, 1)
    ).reshape(-1)


def _run(x: np.ndarray, **kwargs):
    from concourse.bass_utils import run_bass_kernel_spmd

    nc = _get_nc()
    g = _make_g()
    shards = x.reshape(N_CORES, B_PER_CORE, F, E)
    in_maps = [
        {"x": np.ascontiguousarray(shards[c]), "g": g} for c in range(N_CORES)
    ]
    res = run_bass_kernel_spmd(nc, in_maps, core_ids=list(range(N_CORES)), **kwargs)
    out = np.concatenate(
        [_unpermute(np.asarray(res.results[c]["out"])) for c in range(N_CORES)]
    ).astype(np.float32)
    return out, res


def kernel(**inputs) -> np.ndarray:
    x = np.ascontiguousarray(np.asarray(inputs["x"], dtype=np.float32))
    assert x.shape == (B, F, E), x.shape
    out, _ = _run(x)
    return out
